# revision 18
# baseline (speedup 1.0000x reference)
"""Sparse-attention layer on 8 TRN2 NeuronCores (data-parallel over batch).

Reference computation (per batch b):
    q = states @ Wq; k = key @ Wk; v = key @ Wv            [T, H, A]
    alpha[h,q,k] = q.k + bs[q,k]*ksum[k,h]                 (bs = sparse edge bias scatter)
    alpha = alpha/8 - mask*BIG; P = softmax_k(alpha)
    out = (P @ v) @ Wout                                   [T, D]

Device strategy (one batch per core, no collectives):
  - scores are computed TRANSPOSED, S^T[k,q], so the bias term bs[q,k]*ksum[k,h]
    becomes a per-partition scalar multiply -> one fused DVE scalar_tensor_tensor
    (bias apply + PSUM evacuation + bf16 cast in a single pass).
  - exp without max-subtraction (scores are O(20); fp32 exp range is ample);
    mask enters as an additive -30000 before the exp.
  - context matmul carries a fused ones-column producing softmax denominators.
  - output projection consumes ctx^T directly; host transposes the [D,T] result.
"""

import sys

sys.path.insert(0, "/opt/trn_rl_repo")

import ml_dtypes
import numpy as np

import concourse.bass as bass
import concourse.tile as tile
from concourse import bacc, mybir
from concourse.bass_utils import run_bass_kernel_spmd

BF16 = mybir.dt.bfloat16
F32 = mybir.dt.float32
MULT = mybir.AluOpType.mult
ADD = mybir.AluOpType.add
EXP = mybir.ActivationFunctionType.Exp

B, T, D, H, A = 8, 1024, 1024, 16, 64
HA = H * A
P = 128
KD = D // P      # 8 contraction tiles over D
KT = T // P      # 8 tiles over key tokens
NQ = 2           # query-token 512-chunks
NC_ = 512        # free-dim chunk
MASK_NEG = -30000.0

_CACHED_NC = None


def _build_nc():
    nc = bacc.Bacc("TRN2", target_bir_lowering=False, debug=False, num_devices=8)

    xT = nc.dram_tensor("xT", [D, T], BF16, kind="ExternalInput")
    yT = nc.dram_tensor("yT", [D, T], BF16, kind="ExternalInput")
    wq = nc.dram_tensor("wq", [D, HA], BF16, kind="ExternalInput")
    wk = nc.dram_tensor("wk", [D, HA], BF16, kind="ExternalInput")
    wv = nc.dram_tensor("wv", [D, HA], BF16, kind="ExternalInput")
    wks = nc.dram_tensor("wks", [D, H], BF16, kind="ExternalInput")
    wot = nc.dram_tensor("wot", [KD, HA, P], BF16, kind="ExternalInput")
    bsm = nc.dram_tensor("bsm", [T, T], BF16, kind="ExternalInput")
    mneg = nc.dram_tensor("mneg", [T, T], BF16, kind="ExternalInput")
    out = nc.dram_tensor("out", [D, T], F32, kind="ExternalOutput")

    with tile.TileContext(nc) as tc:
        with tc.tile_pool(name="persist", bufs=1) as pp, \
             tc.tile_pool(name="dscr", bufs=1, space="DRAM") as dpool, \
             tc.tile_pool(name="pb", bufs=1) as pb, \
             tc.tile_pool(name="ptmp", bufs=2) as ptmp, \
             tc.tile_pool(name="prst", bufs=2) as prst, \
             tc.tile_pool(name="pblk", bufs=4) as pblk, \
             tc.tile_pool(name="rbp", bufs=2) as rbp, \
             tc.tile_pool(name="po", bufs=2) as po, \
             tc.tile_pool(name="pwom", bufs=2) as pwom, \
             tc.tile_pool(name="sps", bufs=4, space="PSUM") as spsum, \
             tc.tile_pool(name="cps", bufs=2, space="PSUM") as cpsum, \
             tc.tile_pool(name="aps", bufs=2, space="PSUM") as apsum:
            qT = [pp.tile([P, T], BF16, tag=f"qT{i}", name=f"qT{i}")
                  for i in range(KT)]
            kTt = [pp.tile([P, T], BF16, tag=f"kT{i}", name=f"kT{i}")
                   for i in range(KT)]
            v_sb = [pp.tile([P, H, A + 1], BF16, tag=f"v{i}", name=f"v{i}")
                    for i in range(KT)]
            ksum = pp.tile([P, KT * H], F32, tag="ksum", name="ksum")
            ctxT = [pp.tile([P, T], BF16, tag=f"ctx{i}", name=f"ctx{i}")
                    for i in range(KT)]
            rs = pp.tile([4 * H, NC_], F32, tag="rs", name="rs")  # row n*32+h
            rsr = pp.tile([4 * H, NC_], F32, tag="rsr", name="rsr")
            scr = dpool.tile([4 * H, NC_], F32, name="scr")

            # ---- inputs for the key-side projections ----
            pool_y = tc.alloc_tile_pool(name="py", bufs=1)
            yTs = [pool_y.tile([P, T], BF16, tag=f"yTs{i}", name=f"yTs{i}")
                   for i in range(KD)]
            wkss = pool_y.tile([P, KD * H], BF16, tag="wkss", name="wkss")
            for i in range(KD):
                sl = slice(i * P, (i + 1) * P)
                nc.sync.dma_start(yTs[i][:], yT.ap()[sl, :])
                nc.sync.dma_start(wkss[:, i * H:(i + 1) * H], wks.ap()[sl, :])
            # attention-side constants (space is free this early)
            bsm_sb = [pb.tile([P, T], BF16, tag=f"bsm{i}", name=f"bsm{i}")
                      for i in range(KT)]
            mneg_n = [pb.tile([P, KT, NC_], BF16, tag=f"mnegn{n}",
                              name=f"mnegn{n}") for n in range(NQ)]
            for i in range(KT):
                sl = slice(i * P, (i + 1) * P)
                nc.sync.dma_start(bsm_sb[i][:], bsm.ap()[sl, :])
                for n in range(NQ):
                    nc.sync.dma_start(mneg_n[n][:, i, :],
                                      mneg.ap()[sl, n * NC_:(n + 1) * NC_])

            # ---- ksum ----
            for m in range(KT):
                msl = slice(m * P, (m + 1) * P)
                ps = apsum.tile([P, NC_], F32, tag="aps", name="aps")
                for kd in range(KD):
                    nc.tensor.matmul(ps[:, 0:H], yTs[kd][:, msl],
                                     wkss[:, kd * H:(kd + 1) * H],
                                     start=(kd == 0), stop=(kd == KD - 1))
                nc.vector.tensor_copy(ksum[:, m * H:(m + 1) * H], ps[:, 0:H])

            # ---- v ----
            pool_v = tc.alloc_tile_pool(name="pv", bufs=1)
            wvt = [pool_v.tile([P, HA], BF16, tag=f"wvt{i}", name=f"wvt{i}")
                   for i in range(KD)]
            for i in range(KD):
                nc.sync.dma_start(wvt[i][:], wv.ap()[i * P:(i + 1) * P, :])
            for m in range(KT):
                msl = slice(m * P, (m + 1) * P)
                nc.gpsimd.memset(v_sb[m][:, :, A:A + 1], 1.0)
                for n in range(NQ):
                    nsl = slice(n * NC_, (n + 1) * NC_)
                    ps = apsum.tile([P, NC_], F32, tag="aps", name="aps")
                    for kd in range(KD):
                        nc.tensor.matmul(ps[:], yTs[kd][:, msl],
                                         wvt[kd][:, nsl],
                                         start=(kd == 0), stop=(kd == KD - 1))
                    nc.scalar.copy(
                        v_sb[m][:, n * (H // 2):(n + 1) * (H // 2), 0:A],
                        ps[:].rearrange("p (h a) -> p h a", a=A))
            pool_v.release()

            # ---- kT ----
            pool_k = tc.alloc_tile_pool(name="pk", bufs=1)
            wkt = [pool_k.tile([P, HA], BF16, tag=f"wkt{i}", name=f"wkt{i}")
                   for i in range(KD)]
            for i in range(KD):
                nc.sync.dma_start(wkt[i][:], wk.ap()[i * P:(i + 1) * P, :])
            for m in range(KT):
                msl = slice(m * P, (m + 1) * P)
                for n in range(NQ):
                    nsl = slice(n * NC_, (n + 1) * NC_)
                    ps = apsum.tile([P, NC_], F32, tag="aps", name="aps")
                    for kd in range(KD):
                        nc.tensor.matmul(ps[:], wkt[kd][:, msl],
                                         yTs[kd][:, nsl],
                                         start=(kd == 0), stop=(kd == KD - 1))
                    nc.scalar.copy(kTt[m][:, nsl], ps[:])
            pool_k.release()
            pool_y.release()

            # ---- qT (just-in-time per head-pair) ----
            pool_x = tc.alloc_tile_pool(name="px", bufs=1)
            xTs = [pool_x.tile([P, T], BF16, tag=f"xTs{i}", name=f"xTs{i}")
                   for i in range(KD)]
            wqs = [pool_x.tile([P, HA], BF16, tag=f"wqs{i}", name=f"wqs{i}")
                   for i in range(KD)]
            for i in range(KD):
                sl = slice(i * P, (i + 1) * P)
                nc.sync.dma_start(xTs[i][:], xT.ap()[sl, :])
                nc.sync.dma_start(wqs[i][:], wq.ap()[sl, :])

            def emit_qT(m):
                msl = slice(m * P, (m + 1) * P)
                for n in range(NQ):
                    nsl = slice(n * NC_, (n + 1) * NC_)
                    ps = apsum.tile([P, NC_], F32, tag="aps", name="aps")
                    for kd in range(KD):
                        nc.tensor.matmul(ps[:], wqs[kd][:, msl],
                                         xTs[kd][:, nsl],
                                         start=(kd == 0), stop=(kd == KD - 1))
                    nc.scalar.copy(qT[m][:, nsl], ps[:])

            emit_qT(0)

            def emit_scores(hp, n):
                nsl = slice(n * NC_, (n + 1) * NC_)
                pblks = [pblk.tile([P, KT, NC_], BF16, tag="Pblk", name="Pblk")
                         for _ in range(2)]
                s1b = [ptmp.tile([P, KT, NC_], BF16, tag="s1", name="s1")
                       for _ in range(2)]
                for kt in range(KT):
                    for hi in range(2):
                        h = 2 * hp + hi
                        roff = hi * A
                        sps = spsum.tile([P, NC_], F32, tag="sps", name="sps")
                        nc.tensor.matmul(
                            sps[:], kTt[hp][roff:roff + A, kt * P:(kt + 1) * P],
                            qT[hp][roff:roff + A, nsl], start=True, stop=True)
                        nc.vector.scalar_tensor_tensor(
                            s1b[hi][:, kt, :], bsm_sb[kt][:, nsl],
                            ksum[:, kt * H + h:kt * H + h + 1],
                            sps[:], op0=MULT, op1=ADD)
                for hi in range(2):
                    pbk = pblks[hi]
                    for kt in range(KT):
                        nc.vector.tensor_tensor(
                            pbk[:, kt, :], s1b[hi][:, kt, :],
                            mneg_n[n][:, kt, :], op=ADD)
                    nc.scalar.activation(pbk[:], pbk[:], EXP, scale=0.125)
                return pblks

            def emit_ctx(hp, n, pblks):
                nsl = slice(n * NC_, (n + 1) * NC_)
                for hi in range(2):
                    h = 2 * hp + hi
                    roff = hi * A
                    cps = cpsum.tile([A + 1, NC_], F32, tag="cps", name="cps")
                    for kt in range(KT):
                        nc.tensor.matmul(
                            cps[:], v_sb[kt][:, h, :], pblks[hi][:, kt, :],
                            start=(kt == 0), stop=(kt == KT - 1))
                    r = n * 2 * H + h
                    rstage = prst.tile([1, NC_], F32, tag="rstage",
                                       name="rstage")
                    nc.scalar.copy(rstage[:], cps[A:A + 1, :])
                    nc.sync.dma_start(rs[r:r + 1, :], rstage[:])
                    nc.scalar.copy(ctxT[hp][roff:roff + A, nsl], cps[0:A, :])

            def emit_out_half(n):
                nsl = slice(n * NC_, (n + 1) * NC_)
                rsl = slice(n * 2 * H, n * 2 * H + H)
                nc.vector.reciprocal(rsr[rsl, :], rs[rsl, :])
                nc.sync.dma_start(scr[rsl, :], rsr[rsl, :])
                for hp in range(H // 2):
                    r0 = n * 2 * H + 2 * hp
                    r1 = n * 2 * H + 2 * hp + 1
                    rb = rbp.tile([P, NC_], F32, tag="rb", name="rb")
                    src0 = bass.AP(scr[:].tensor, scr[:].offset + r0 * NC_,
                                   [[0, A], [1, NC_]])
                    src1 = bass.AP(scr[:].tensor, scr[:].offset + r1 * NC_,
                                   [[0, A], [1, NC_]])
                    nc.sync.dma_start(rb[0:A, :], src0)
                    nc.sync.dma_start(rb[A:P, :], src1)
                    nc.vector.tensor_tensor(ctxT[hp][:, nsl],
                                            ctxT[hp][:, nsl], rb[:], op=MULT)
                for m in range(KD):
                    msl = slice(m * P, (m + 1) * P)
                    wom = pwom.tile([P, KD, P], BF16, tag="wom", name="wom")
                    for kd in range(KD):
                        nc.sync.dma_start(
                            wom[:, kd, :],
                            wot.ap()[m, kd * P:(kd + 1) * P, :])
                    ps = apsum.tile([P, NC_], F32, tag="aps", name="aps")
                    for kt in range(KT):
                        nc.tensor.matmul(ps[:], wom[:, kt, :],
                                         ctxT[kt][:, nsl],
                                         start=(kt == 0), stop=(kt == KT - 1))
                    osb = po.tile([P, NC_], F32, tag="osb", name="osb")
                    nc.scalar.copy(osb[:], ps[:])
                    nc.sync.dma_start(out.ap()[msl, nsl], osb[:])

            iters = [(n, hp) for n in range(NQ) for hp in range(H // 2)]
            pending = []
            for i, (n, hp) in enumerate(iters):
                if len(pending) >= 2:
                    emit_ctx(*pending.pop(0))
                pblks = emit_scores(hp, n)
                pending.append((hp, n, pblks))
                if n == 0 and hp < H // 2 - 1:
                    emit_qT(hp + 1)
                if n == 0 and hp == H // 2 - 1:
                    pool_x.release()
                if i == 9:
                    # all (n=0) ctx done two pops ago; overlap first out-half
                    emit_out_half(0)
            for it in pending:
                emit_ctx(*it)
            emit_out_half(1)

    nc.compile()
    return nc


def _get_nc():
    global _CACHED_NC
    if _CACHED_NC is None:
        _CACHED_NC = _build_nc()
    return _CACHED_NC


def _prep_inputs(states, key_states, masks, attention_bias, Wq, Wk, Wv, Wout,
                 bias_embs, bias_scalar):
    bf = ml_dtypes.bfloat16
    states = np.asarray(states, dtype=np.float32)
    key_states = np.asarray(key_states, dtype=np.float32)
    masks = np.asarray(masks, dtype=np.float32)
    ab = np.asarray(attention_bias)
    Wq = np.asarray(Wq, dtype=np.float32).reshape(D, HA)
    Wk3 = np.asarray(Wk, dtype=np.float32)
    Wv = np.asarray(Wv, dtype=np.float32).reshape(D, HA)
    Wout = np.asarray(Wout, dtype=np.float32).reshape(HA, D)
    bias_embs = np.asarray(bias_embs, dtype=np.float32)
    bias_scalar = np.asarray(bias_scalar, dtype=np.float32)

    bvals = (bias_embs[ab[:, 0]] @ bias_scalar)[:, 0]          # [E]
    wq_b = np.ascontiguousarray(Wq).astype(bf)
    wk_b = np.ascontiguousarray(Wk3.reshape(D, HA)).astype(bf)
    wv_b = np.ascontiguousarray(Wv).astype(bf)
    wks_b = np.ascontiguousarray(Wk3.sum(axis=2)).astype(bf)   # [D, H]
    wot_b = np.ascontiguousarray(
        Wout.reshape(HA, KD, P).transpose(1, 0, 2)).astype(bf)  # [m, HA, 128]

    in_maps = []
    for b in range(B):
        bs = np.zeros((T, T), dtype=np.float32)
        sel = ab[:, 1] == b
        bs[ab[sel, 2], ab[sel, 3]] = bvals[sel]                # last write wins
        in_maps.append({
            "xT": np.ascontiguousarray(states[b].T).astype(bf),
            "yT": np.ascontiguousarray(key_states[b].T).astype(bf),
            "wq": wq_b, "wk": wk_b, "wv": wv_b, "wks": wks_b, "wot": wot_b,
            "bsm": np.ascontiguousarray(bs.T).astype(bf),
            "mneg": np.ascontiguousarray(masks[b].T * MASK_NEG).astype(bf),
        })
    return in_maps


def kernel(**inputs) -> np.ndarray:
    nc = _get_nc()
    in_maps = _prep_inputs(**inputs)
    res = run_bass_kernel_spmd(nc, in_maps, core_ids=list(range(8)))
    out = np.empty((B, T, D), dtype=np.float32)
    for b in range(B):
        out[b] = res.results[b]["out"].T
    return out


# revision 22
# speedup vs baseline: 1.1253x; 1.1253x over previous
"""Sparse-attention layer on 8 TRN2 NeuronCores (data-parallel over batch).

Reference computation (per batch b):
    q = states @ Wq; k = key @ Wk; v = key @ Wv            [T, H, A]
    alpha[h,q,k] = q.k + bs[q,k]*ksum[k,h]                 (bs = sparse edge bias scatter)
    alpha = alpha/8 - mask*BIG; P = softmax_k(alpha)
    out = (P @ v) @ Wout                                   [T, D]

Device strategy (one batch per core, no collectives):
  - scores are computed TRANSPOSED, S^T[k,q], so the bias term bs[q,k]*ksum[k,h]
    becomes a per-partition scalar multiply -> one fused DVE scalar_tensor_tensor
    (bias apply + PSUM evacuation + bf16 cast in a single pass).
  - exp without max-subtraction (scores are O(20); fp32 exp range is ample);
    mask enters as an additive -30000 before the exp.
  - context matmul carries a fused ones-column producing softmax denominators;
    per-iteration ctx bursts (no DVE deps) keep the PE clock gate warm.
  - projections are streamed just-in-time inside the attention loop so the
    DVE (the critical engine) starts within ~15us of kernel start.
  - output projection for the first query half overlaps the second half's
    attention; host transposes the [D,T] result back.
"""

import sys

sys.path.insert(0, "/opt/trn_rl_repo")

import ml_dtypes
import numpy as np

import concourse.bass as bass
import concourse.tile as tile
from concourse import bacc, mybir
from concourse.bass_utils import run_bass_kernel_spmd

BF16 = mybir.dt.bfloat16
F32 = mybir.dt.float32
MULT = mybir.AluOpType.mult
ADD = mybir.AluOpType.add
EXP = mybir.ActivationFunctionType.Exp

B, T, D, H, A = 8, 1024, 1024, 16, 64
HA = H * A
P = 128
KD = D // P      # contraction tiles over D
KT = T // P      # tiles over key tokens
NQ = 2           # query-token 512-chunks
NC_ = 512
MASK_NEG = -30000.0

_CACHED_NC = None


def _build_nc():
    nc = bacc.Bacc("TRN2", target_bir_lowering=False, debug=False, num_devices=8)

    xT = nc.dram_tensor("xT", [D, T], BF16, kind="ExternalInput")
    yT = nc.dram_tensor("yT", [D, T], BF16, kind="ExternalInput")
    wqt = nc.dram_tensor("wqt", [KD, D, P], BF16, kind="ExternalInput")
    wkt_d = nc.dram_tensor("wkt", [KD, D, P], BF16, kind="ExternalInput")
    wv = nc.dram_tensor("wv", [D, HA], BF16, kind="ExternalInput")
    wks = nc.dram_tensor("wks", [D, H], BF16, kind="ExternalInput")
    wot = nc.dram_tensor("wot", [KD, HA, P], BF16, kind="ExternalInput")
    bsm = nc.dram_tensor("bsm", [T, T], BF16, kind="ExternalInput")
    mneg = nc.dram_tensor("mneg", [T, T], BF16, kind="ExternalInput")
    out = nc.dram_tensor("out", [D, T], F32, kind="ExternalOutput")

    with tile.TileContext(nc) as tc:
        with tc.tile_pool(name="persist", bufs=1) as pp, \
             tc.tile_pool(name="dscr", bufs=1, space="DRAM") as dpool, \
             tc.tile_pool(name="pb", bufs=1) as pb, \
             tc.tile_pool(name="ptmp", bufs=6) as ptmp, \
             tc.tile_pool(name="prst", bufs=2) as prst, \
             tc.tile_pool(name="pblk", bufs=4) as pblk, \
             tc.tile_pool(name="pwqk", bufs=2) as pwqk, \
             tc.tile_pool(name="pqk", bufs=3) as pqk, \
             tc.tile_pool(name="rbp", bufs=2) as rbp, \
             tc.tile_pool(name="po", bufs=2) as po, \
             tc.tile_pool(name="pwom", bufs=2) as pwom, \
             tc.tile_pool(name="sps", bufs=4, space="PSUM") as spsum, \
             tc.tile_pool(name="cps", bufs=2, space="PSUM") as cpsum, \
             tc.tile_pool(name="aps", bufs=2, space="PSUM") as apsum:
            v_sb = [pp.tile([P, H, A + 1], BF16, tag=f"v{i}", name=f"v{i}")
                    for i in range(KT)]
            ksum = pp.tile([P, KT * H], F32, tag="ksum", name="ksum")
            ctxT = [pp.tile([P, T], BF16, tag=f"ctx{i}", name=f"ctx{i}")
                    for i in range(KT)]
            rs = pp.tile([4 * H, NC_], F32, tag="rs", name="rs")  # row n*32+h
            rsr = pp.tile([4 * H, NC_], F32, tag="rsr", name="rsr")
            scr = dpool.tile([4 * H, NC_], F32, name="scr")

            # ---- loads for the prologue (key side first) ----
            pool_y = tc.alloc_tile_pool(name="py", bufs=1)
            yTs = [pool_y.tile([P, T], BF16, tag=f"yTs{i}", name=f"yTs{i}")
                   for i in range(KD)]
            wkss = pool_y.tile([P, KD * H], BF16, tag="wkss", name="wkss")
            for i in range(KD):
                sl = slice(i * P, (i + 1) * P)
                nc.sync.dma_start(yTs[i][:], yT.ap()[sl, :])
                nc.sync.dma_start(wkss[:, i * H:(i + 1) * H], wks.ap()[sl, :])
            pool_x = tc.alloc_tile_pool(name="px", bufs=1)
            xTs = [pool_x.tile([P, T], BF16, tag=f"xTs{i}", name=f"xTs{i}")
                   for i in range(KD)]
            for i in range(KD):
                nc.sync.dma_start(xTs[i][:], xT.ap()[i * P:(i + 1) * P, :])
            bsm_sb = [pb.tile([P, T], BF16, tag=f"bsm{i}", name=f"bsm{i}")
                      for i in range(KT)]
            mneg_n = [pb.tile([P, KT, NC_], BF16, tag=f"mnegn{n}",
                              name=f"mnegn{n}") for n in range(NQ)]
            for i in range(KT):
                sl = slice(i * P, (i + 1) * P)
                nc.sync.dma_start(bsm_sb[i][:], bsm.ap()[sl, :])
                for n in range(NQ):
                    nc.sync.dma_start(mneg_n[n][:, i, :],
                                      mneg.ap()[sl, n * NC_:(n + 1) * NC_])
            pool_v = tc.alloc_tile_pool(name="pv", bufs=1)
            wvt = [pool_v.tile([P, HA], BF16, tag=f"wvt{i}", name=f"wvt{i}")
                   for i in range(KD)]
            for i in range(KD):
                nc.sync.dma_start(wvt[i][:], wv.ap()[i * P:(i + 1) * P, :])

            # ---- ksum (needed by every score tile) ----
            for m in range(KT):
                msl = slice(m * P, (m + 1) * P)
                ps = apsum.tile([P, NC_], F32, tag="aps", name="aps")
                for kd in range(KD):
                    nc.tensor.matmul(ps[:, 0:H], yTs[kd][:, msl],
                                     wkss[:, kd * H:(kd + 1) * H],
                                     start=(kd == 0), stop=(kd == KD - 1))
                nc.vector.tensor_copy(ksum[:, m * H:(m + 1) * H], ps[:, 0:H])

            def emit_kT(m):
                kTr = pqk.tile([P, T], BF16, tag="kTr", name="kTr")
                wkm = pwqk.tile([P, KD, P], BF16, tag="wkm", name="wkm")
                for kd in range(KD):
                    nc.sync.dma_start(wkm[:, kd, :],
                                      wkt_d.ap()[m, kd * P:(kd + 1) * P, :])
                for n in range(NQ):
                    nsl = slice(n * NC_, (n + 1) * NC_)
                    ps = apsum.tile([P, NC_], F32, tag="aps", name="aps")
                    for kd in range(KD):
                        nc.tensor.matmul(ps[:], wkm[:, kd, :],
                                         yTs[kd][:, nsl],
                                         start=(kd == 0), stop=(kd == KD - 1))
                    nc.scalar.copy(kTr[:, nsl], ps[:])
                return kTr

            def emit_qT(m):
                qTr = pqk.tile([P, T], BF16, tag="qTr", name="qTr")
                wqm = pwqk.tile([P, KD, P], BF16, tag="wqm", name="wqm")
                for kd in range(KD):
                    nc.sync.dma_start(wqm[:, kd, :],
                                      wqt.ap()[m, kd * P:(kd + 1) * P, :])
                for n in range(NQ):
                    nsl = slice(n * NC_, (n + 1) * NC_)
                    ps = apsum.tile([P, NC_], F32, tag="aps", name="aps")
                    for kd in range(KD):
                        nc.tensor.matmul(ps[:], wqm[:, kd, :],
                                         xTs[kd][:, nsl],
                                         start=(kd == 0), stop=(kd == KD - 1))
                    nc.scalar.copy(qTr[:, nsl], ps[:])
                return qTr

            def emit_v(m):
                msl = slice(m * P, (m + 1) * P)
                nc.gpsimd.memset(v_sb[m][:, :, A:A + 1], 1.0)
                for n in range(NQ):
                    nsl = slice(n * NC_, (n + 1) * NC_)
                    ps = apsum.tile([P, NC_], F32, tag="aps", name="aps")
                    for kd in range(KD):
                        nc.tensor.matmul(ps[:], yTs[kd][:, msl],
                                         wvt[kd][:, nsl],
                                         start=(kd == 0), stop=(kd == KD - 1))
                    nc.scalar.copy(
                        v_sb[m][:, n * (H // 2):(n + 1) * (H // 2), 0:A],
                        ps[:].rearrange("p (h a) -> p h a", a=A))

            def emit_scores(hp, n, kTr, qTr):
                nsl = slice(n * NC_, (n + 1) * NC_)
                pblks = [pblk.tile([P, KT, NC_], BF16, tag="Pblk", name="Pblk")
                         for _ in range(2)]
                for kt in range(KT):
                    for hi in range(2):
                        h = 2 * hp + hi
                        roff = hi * A
                        sps = spsum.tile([P, NC_], F32, tag="sps", name="sps")
                        nc.tensor.matmul(
                            sps[:], kTr[roff:roff + A, kt * P:(kt + 1) * P],
                            qTr[roff:roff + A, nsl], start=True, stop=True)
                        s1 = ptmp.tile([P, NC_], BF16, tag="s1", name="s1")
                        nc.vector.scalar_tensor_tensor(
                            s1[:], bsm_sb[kt][:, nsl],
                            ksum[:, kt * H + h:kt * H + h + 1],
                            sps[:], op0=MULT, op1=ADD)
                        nc.vector.tensor_tensor(
                            pblks[hi][:, kt, :], s1[:],
                            mneg_n[n][:, kt, :], op=ADD)
                for hi in range(2):
                    nc.scalar.activation(pblks[hi][:], pblks[hi][:], EXP,
                                         scale=0.125)
                return pblks

            def emit_ctx(hp, n, pblks):
                nsl = slice(n * NC_, (n + 1) * NC_)
                for hi in range(2):
                    h = 2 * hp + hi
                    roff = hi * A
                    cps = cpsum.tile([A + 1, NC_], F32, tag="cps", name="cps")
                    for kt in range(KT):
                        nc.tensor.matmul(
                            cps[:], v_sb[kt][:, h, :], pblks[hi][:, kt, :],
                            start=(kt == 0), stop=(kt == KT - 1))
                    r = n * 2 * H + h
                    rstage = prst.tile([1, NC_], F32, tag="rstage",
                                       name="rstage")
                    nc.scalar.copy(rstage[:], cps[A:A + 1, :])
                    nc.sync.dma_start(rs[r:r + 1, :], rstage[:])
                    nc.scalar.copy(ctxT[hp][roff:roff + A, nsl], cps[0:A, :])

            def emit_out_half(n):
                nsl = slice(n * NC_, (n + 1) * NC_)
                rsl = slice(n * 2 * H, n * 2 * H + H)
                nc.vector.reciprocal(rsr[rsl, :], rs[rsl, :])
                nc.sync.dma_start(scr[rsl, :], rsr[rsl, :])
                for hp in range(H // 2):
                    r0 = n * 2 * H + 2 * hp
                    r1 = n * 2 * H + 2 * hp + 1
                    rb = rbp.tile([P, NC_], F32, tag="rb", name="rb")
                    src0 = bass.AP(scr[:].tensor, scr[:].offset + r0 * NC_,
                                   [[0, A], [1, NC_]])
                    src1 = bass.AP(scr[:].tensor, scr[:].offset + r1 * NC_,
                                   [[0, A], [1, NC_]])
                    nc.sync.dma_start(rb[0:A, :], src0)
                    nc.sync.dma_start(rb[A:P, :], src1)
                    nc.vector.tensor_tensor(ctxT[hp][:, nsl],
                                            ctxT[hp][:, nsl], rb[:], op=MULT)
                for m in range(KD):
                    msl = slice(m * P, (m + 1) * P)
                    wom = pwom.tile([P, KD, P], BF16, tag="wom", name="wom")
                    for kd in range(KD):
                        nc.sync.dma_start(
                            wom[:, kd, :],
                            wot.ap()[m, kd * P:(kd + 1) * P, :])
                    ps = apsum.tile([P, NC_], F32, tag="aps", name="aps")
                    for kt in range(KT):
                        nc.tensor.matmul(ps[:], wom[:, kt, :],
                                         ctxT[kt][:, nsl],
                                         start=(kt == 0), stop=(kt == KT - 1))
                    osb = po.tile([P, NC_], F32, tag="osb", name="osb")
                    nc.scalar.copy(osb[:], ps[:])
                    nc.sync.dma_start(out.ap()[msl, nsl], osb[:])

            cur_k = emit_kT(0)
            cur_q = emit_qT(0)

            pending = []
            for hp in range(H // 2):
                for n in range(NQ):
                    if len(pending) >= 2:
                        emit_ctx(*pending.pop(0))
                    pblks = emit_scores(hp, n, cur_k, cur_q)
                    pending.append((hp, n, pblks))
                    if n == 0:
                        if hp == 0:
                            for m in range(4):
                                emit_v(m)
                    else:
                        if hp == 0:
                            for m in range(4, KT):
                                emit_v(m)
                            pool_v.release()
                        if hp < H // 2 - 1:
                            cur_k = emit_kT(hp + 1)
                            cur_q = emit_qT(hp + 1)
                        if hp == H // 2 - 1:
                            pool_x.release()
                            pool_y.release()
            for it in pending:
                emit_ctx(*it)
            emit_out_half(0)
            emit_out_half(1)

    nc.compile()
    return nc


def _get_nc():
    global _CACHED_NC
    if _CACHED_NC is None:
        _CACHED_NC = _build_nc()
    return _CACHED_NC


def _prep_inputs(states, key_states, masks, attention_bias, Wq, Wk, Wv, Wout,
                 bias_embs, bias_scalar):
    bf = ml_dtypes.bfloat16
    states = np.asarray(states, dtype=np.float32)
    key_states = np.asarray(key_states, dtype=np.float32)
    masks = np.asarray(masks, dtype=np.float32)
    ab = np.asarray(attention_bias)
    Wq2 = np.asarray(Wq, dtype=np.float32).reshape(D, HA)
    Wk3 = np.asarray(Wk, dtype=np.float32)
    Wv2 = np.asarray(Wv, dtype=np.float32).reshape(D, HA)
    Wout2 = np.asarray(Wout, dtype=np.float32).reshape(HA, D)
    bias_embs = np.asarray(bias_embs, dtype=np.float32)
    bias_scalar = np.asarray(bias_scalar, dtype=np.float32)

    bvals = (bias_embs[ab[:, 0]] @ bias_scalar)[:, 0]          # [E]
    # weight tile layouts: [m, D, 128] column blocks
    wqt_b = np.ascontiguousarray(
        Wq2.reshape(D, KD, P).transpose(1, 0, 2)).astype(bf)
    wkt_b = np.ascontiguousarray(
        Wk3.reshape(D, HA).reshape(D, KD, P).transpose(1, 0, 2)).astype(bf)
    wv_b = np.ascontiguousarray(Wv2).astype(bf)
    wks_b = np.ascontiguousarray(Wk3.sum(axis=2)).astype(bf)   # [D, H]
    wot_b = np.ascontiguousarray(
        Wout2.reshape(HA, KD, P).transpose(1, 0, 2)).astype(bf)

    in_maps = []
    for b in range(B):
        bs = np.zeros((T, T), dtype=np.float32)
        sel = ab[:, 1] == b
        bs[ab[sel, 2], ab[sel, 3]] = bvals[sel]                # last write wins
        in_maps.append({
            "xT": np.ascontiguousarray(states[b].T).astype(bf),
            "yT": np.ascontiguousarray(key_states[b].T).astype(bf),
            "wqt": wqt_b, "wkt": wkt_b, "wv": wv_b, "wks": wks_b,
            "wot": wot_b,
            "bsm": np.ascontiguousarray(bs.T).astype(bf),
            "mneg": np.ascontiguousarray(masks[b].T * MASK_NEG).astype(bf),
        })
    return in_maps


def kernel(**inputs) -> np.ndarray:
    nc = _get_nc()
    in_maps = _prep_inputs(**inputs)
    res = run_bass_kernel_spmd(nc, in_maps, core_ids=list(range(8)))
    out = np.empty((B, T, D), dtype=np.float32)
    for b in range(B):
        out[b] = res.results[b]["out"].T
    return out


# revision 23
# speedup vs baseline: 1.2666x; 1.1255x over previous
"""Sparse-attention layer on 8 TRN2 NeuronCores (data-parallel over batch).

Reference computation (per batch b):
    q = states @ Wq; k = key @ Wk; v = key @ Wv            [T, H, A]
    alpha[h,q,k] = q.k + bs[q,k]*ksum[k,h]                 (bs = sparse edge bias scatter)
    alpha = alpha/8 - mask*BIG; P = softmax_k(alpha)
    out = (P @ v) @ Wout                                   [T, D]

Device strategy (one batch per core, no collectives):
  - scores are computed TRANSPOSED, S^T[k,q], so the bias term bs[q,k]*ksum[k,h]
    becomes a per-partition scalar multiply -> one fused DVE scalar_tensor_tensor
    (bias apply + PSUM evacuation + bf16 cast in a single pass).
  - exp without max-subtraction (scores are O(20); fp32 exp range is ample);
    mask enters as an additive -30000 before the exp.
  - context matmul carries a fused ones-column producing softmax denominators;
    per-iteration ctx bursts (no DVE deps) keep the PE clock gate warm.
  - projections are streamed just-in-time inside the attention loop so the
    DVE (the critical engine) starts within ~15us of kernel start.
  - output projection for the first query half overlaps the second half's
    attention; host transposes the [D,T] result back.
"""

import sys

sys.path.insert(0, "/opt/trn_rl_repo")

import ml_dtypes
import numpy as np

import concourse.bass as bass
import concourse.tile as tile
from concourse import bacc, mybir
from concourse.bass_utils import run_bass_kernel_spmd

BF16 = mybir.dt.bfloat16
F32 = mybir.dt.float32
MULT = mybir.AluOpType.mult
ADD = mybir.AluOpType.add
EXP = mybir.ActivationFunctionType.Exp

B, T, D, H, A = 8, 1024, 1024, 16, 64
HA = H * A
P = 128
KD = D // P      # contraction tiles over D
KT = T // P      # tiles over key tokens
NQ = 2           # query-token 512-chunks
NC_ = 512
MASK_NEG = -30000.0

_CACHED_NC = None


def _build_nc():
    nc = bacc.Bacc("TRN2", target_bir_lowering=False, debug=False, num_devices=8)

    xT = nc.dram_tensor("xT", [D, T], BF16, kind="ExternalInput")
    yT = nc.dram_tensor("yT", [D, T], BF16, kind="ExternalInput")
    wqt = nc.dram_tensor("wqt", [KD, D, P], BF16, kind="ExternalInput")
    wkt_d = nc.dram_tensor("wkt", [KD, D, P], BF16, kind="ExternalInput")
    wv = nc.dram_tensor("wv", [D, HA], BF16, kind="ExternalInput")
    wks = nc.dram_tensor("wks", [D, H], BF16, kind="ExternalInput")
    wot = nc.dram_tensor("wot", [KD, HA, P], BF16, kind="ExternalInput")
    bsm = nc.dram_tensor("bsm", [T, T], BF16, kind="ExternalInput")
    mneg = nc.dram_tensor("mneg", [T, T], BF16, kind="ExternalInput")
    out = nc.dram_tensor("out", [D, T], F32, kind="ExternalOutput")

    with tile.TileContext(nc) as tc:
        with tc.tile_pool(name="persist", bufs=1) as pp, \
             tc.tile_pool(name="dscr", bufs=1, space="DRAM") as dpool, \
             tc.tile_pool(name="pb", bufs=1) as pb, \
             tc.tile_pool(name="ptmp", bufs=6) as ptmp, \
             tc.tile_pool(name="prst", bufs=2) as prst, \
             tc.tile_pool(name="pblk", bufs=4) as pblk, \
             tc.tile_pool(name="pwqk", bufs=2) as pwqk, \
             tc.tile_pool(name="pqk", bufs=3) as pqk, \
             tc.tile_pool(name="rbp", bufs=4) as rbp, \
             tc.tile_pool(name="po", bufs=2) as po, \
             tc.tile_pool(name="pwom", bufs=2) as pwom, \
             tc.tile_pool(name="sps", bufs=4, space="PSUM") as spsum, \
             tc.tile_pool(name="cps", bufs=2, space="PSUM") as cpsum, \
             tc.tile_pool(name="aps", bufs=2, space="PSUM") as apsum:
            v_sb = [pp.tile([P, H, A + 1], BF16, tag=f"v{i}", name=f"v{i}")
                    for i in range(KT)]
            ksum = pp.tile([P, KT * H], F32, tag="ksum", name="ksum")
            ctxT = [pp.tile([P, T], BF16, tag=f"ctx{i}", name=f"ctx{i}")
                    for i in range(KT)]
            rs = pp.tile([4 * H, NC_], F32, tag="rs", name="rs")  # row n*32+h
            rsr = pp.tile([4 * H, NC_], F32, tag="rsr", name="rsr")
            scr = dpool.tile([4 * H, NC_], F32, name="scr")

            # ---- loads for the prologue (key side first) ----
            pool_y = tc.alloc_tile_pool(name="py", bufs=1)
            yTs = [pool_y.tile([P, T], BF16, tag=f"yTs{i}", name=f"yTs{i}")
                   for i in range(KD)]
            wkss = pool_y.tile([P, KD * H], BF16, tag="wkss", name="wkss")
            for i in range(KD):
                sl = slice(i * P, (i + 1) * P)
                nc.sync.dma_start(yTs[i][:], yT.ap()[sl, :])
                nc.sync.dma_start(wkss[:, i * H:(i + 1) * H], wks.ap()[sl, :])
            pool_x = tc.alloc_tile_pool(name="px", bufs=1)
            xTs = [pool_x.tile([P, T], BF16, tag=f"xTs{i}", name=f"xTs{i}")
                   for i in range(KD)]
            for i in range(KD):
                nc.sync.dma_start(xTs[i][:], xT.ap()[i * P:(i + 1) * P, :])
            bsm_sb = [pb.tile([P, T], BF16, tag=f"bsm{i}", name=f"bsm{i}")
                      for i in range(KT)]
            mneg_n = [pb.tile([P, KT, NC_], BF16, tag=f"mnegn{n}",
                              name=f"mnegn{n}") for n in range(NQ)]
            for i in range(KT):
                sl = slice(i * P, (i + 1) * P)
                nc.sync.dma_start(bsm_sb[i][:], bsm.ap()[sl, :])
                for n in range(NQ):
                    nc.sync.dma_start(mneg_n[n][:, i, :],
                                      mneg.ap()[sl, n * NC_:(n + 1) * NC_])
            pool_v = tc.alloc_tile_pool(name="pv", bufs=1)
            wvt = [pool_v.tile([P, HA], BF16, tag=f"wvt{i}", name=f"wvt{i}")
                   for i in range(KD)]
            for i in range(KD):
                nc.sync.dma_start(wvt[i][:], wv.ap()[i * P:(i + 1) * P, :])

            # ---- ksum (needed by every score tile) ----
            for m in range(KT):
                msl = slice(m * P, (m + 1) * P)
                ps = apsum.tile([P, NC_], F32, tag="aps", name="aps")
                for kd in range(KD):
                    nc.tensor.matmul(ps[:, 0:H], yTs[kd][:, msl],
                                     wkss[:, kd * H:(kd + 1) * H],
                                     start=(kd == 0), stop=(kd == KD - 1))
                nc.vector.tensor_copy(ksum[:, m * H:(m + 1) * H], ps[:, 0:H])

            def emit_kT(m):
                kTr = pqk.tile([P, T], BF16, tag="kTr", name="kTr")
                wkm = pwqk.tile([P, KD, P], BF16, tag="wkm", name="wkm")
                for kd in range(KD):
                    nc.sync.dma_start(wkm[:, kd, :],
                                      wkt_d.ap()[m, kd * P:(kd + 1) * P, :])
                for n in range(NQ):
                    nsl = slice(n * NC_, (n + 1) * NC_)
                    ps = apsum.tile([P, NC_], F32, tag="aps", name="aps")
                    for kd in range(KD):
                        nc.tensor.matmul(ps[:], wkm[:, kd, :],
                                         yTs[kd][:, nsl],
                                         start=(kd == 0), stop=(kd == KD - 1))
                    nc.scalar.copy(kTr[:, nsl], ps[:])
                return kTr

            def emit_qT(m):
                qTr = pqk.tile([P, T], BF16, tag="qTr", name="qTr")
                wqm = pwqk.tile([P, KD, P], BF16, tag="wqm", name="wqm")
                for kd in range(KD):
                    nc.sync.dma_start(wqm[:, kd, :],
                                      wqt.ap()[m, kd * P:(kd + 1) * P, :])
                for n in range(NQ):
                    nsl = slice(n * NC_, (n + 1) * NC_)
                    ps = apsum.tile([P, NC_], F32, tag="aps", name="aps")
                    for kd in range(KD):
                        nc.tensor.matmul(ps[:], wqm[:, kd, :],
                                         xTs[kd][:, nsl],
                                         start=(kd == 0), stop=(kd == KD - 1))
                    nc.scalar.copy(qTr[:, nsl], ps[:])
                return qTr

            def emit_v(m):
                msl = slice(m * P, (m + 1) * P)
                nc.gpsimd.memset(v_sb[m][:, :, A:A + 1], 1.0)
                for n in range(NQ):
                    nsl = slice(n * NC_, (n + 1) * NC_)
                    ps = apsum.tile([P, NC_], F32, tag="aps", name="aps")
                    for kd in range(KD):
                        nc.tensor.matmul(ps[:], yTs[kd][:, msl],
                                         wvt[kd][:, nsl],
                                         start=(kd == 0), stop=(kd == KD - 1))
                    nc.scalar.copy(
                        v_sb[m][:, n * (H // 2):(n + 1) * (H // 2), 0:A],
                        ps[:].rearrange("p (h a) -> p h a", a=A))

            def emit_scores(hp, n, kTr, qTr):
                nsl = slice(n * NC_, (n + 1) * NC_)
                pblks = [pblk.tile([P, KT, NC_], BF16, tag="Pblk", name="Pblk")
                         for _ in range(2)]
                for kt in range(KT):
                    for hi in range(2):
                        h = 2 * hp + hi
                        roff = hi * A
                        sps = spsum.tile([P, NC_], F32, tag="sps", name="sps")
                        nc.tensor.matmul(
                            sps[:], kTr[roff:roff + A, kt * P:(kt + 1) * P],
                            qTr[roff:roff + A, nsl], start=True, stop=True)
                        s1 = ptmp.tile([P, NC_], BF16, tag="s1", name="s1")
                        nc.vector.scalar_tensor_tensor(
                            s1[:], bsm_sb[kt][:, nsl],
                            ksum[:, kt * H + h:kt * H + h + 1],
                            sps[:], op0=MULT, op1=ADD)
                        nc.vector.tensor_tensor(
                            pblks[hi][:, kt, :], s1[:],
                            mneg_n[n][:, kt, :], op=ADD)
                for hi in range(2):
                    nc.scalar.activation(pblks[hi][:], pblks[hi][:], EXP,
                                         scale=0.125)
                return pblks

            def emit_ctx(hp, n, pblks):
                nsl = slice(n * NC_, (n + 1) * NC_)
                for hi in range(2):
                    h = 2 * hp + hi
                    roff = hi * A
                    cps = cpsum.tile([A + 1, NC_], F32, tag="cps", name="cps")
                    for kt in range(KT):
                        nc.tensor.matmul(
                            cps[:], v_sb[kt][:, h, :], pblks[hi][:, kt, :],
                            start=(kt == 0), stop=(kt == KT - 1))
                    r = n * 2 * H + h
                    rstage = prst.tile([1, NC_], F32, tag="rstage",
                                       name="rstage")
                    nc.scalar.copy(rstage[:], cps[A:A + 1, :])
                    nc.sync.dma_start(rs[r:r + 1, :], rstage[:])
                    nc.scalar.copy(ctxT[hp][roff:roff + A, nsl], cps[0:A, :])

            def emit_out_tail():
                for n in range(NQ):
                    rsl = slice(n * 2 * H, n * 2 * H + H)
                    nc.vector.reciprocal(rsr[rsl, :], rs[rsl, :])
                    nc.sync.dma_start(scr[rsl, :], rsr[rsl, :])
                # normalize all chunks; rb cast to bf16 so the multiply is 2x
                for n in range(NQ):
                    nsl = slice(n * NC_, (n + 1) * NC_)
                    for hp in range(H // 2):
                        r0 = n * 2 * H + 2 * hp
                        r1 = n * 2 * H + 2 * hp + 1
                        rb = rbp.tile([P, NC_], BF16, tag="rb", name="rb")
                        src0 = bass.AP(scr[:].tensor, scr[:].offset + r0 * NC_,
                                       [[0, A], [1, NC_]])
                        src1 = bass.AP(scr[:].tensor, scr[:].offset + r1 * NC_,
                                       [[0, A], [1, NC_]])
                        nc.gpsimd.dma_start(rb[0:A, :], src0)
                        nc.gpsimd.dma_start(rb[A:P, :], src1)
                        nc.vector.tensor_tensor(ctxT[hp][:, nsl],
                                                ctxT[hp][:, nsl], rb[:],
                                                op=MULT)
                for m in range(KD):
                    msl = slice(m * P, (m + 1) * P)
                    wom = pwom.tile([P, KD, P], BF16, tag="wom", name="wom")
                    for kd in range(KD):
                        nc.sync.dma_start(
                            wom[:, kd, :],
                            wot.ap()[m, kd * P:(kd + 1) * P, :])
                    for n in range(NQ):
                        nsl = slice(n * NC_, (n + 1) * NC_)
                        ps = apsum.tile([P, NC_], F32, tag="aps", name="aps")
                        for kt in range(KT):
                            nc.tensor.matmul(ps[:], wom[:, kt, :],
                                             ctxT[kt][:, nsl],
                                             start=(kt == 0),
                                             stop=(kt == KT - 1))
                        osb = po.tile([P, NC_], F32, tag="osb", name="osb")
                        nc.scalar.copy(osb[:], ps[:])
                        nc.sync.dma_start(out.ap()[msl, nsl], osb[:])

            cur_k = emit_kT(0)
            cur_q = emit_qT(0)

            pending = []
            for hp in range(H // 2):
                for n in range(NQ):
                    if len(pending) >= 2:
                        emit_ctx(*pending.pop(0))
                    pblks = emit_scores(hp, n, cur_k, cur_q)
                    pending.append((hp, n, pblks))
                    if n == 0:
                        if hp == 0:
                            for m in range(4):
                                emit_v(m)
                    else:
                        if hp == 0:
                            for m in range(4, KT):
                                emit_v(m)
                            pool_v.release()
                        if hp < H // 2 - 1:
                            cur_k = emit_kT(hp + 1)
                            cur_q = emit_qT(hp + 1)
                        if hp == H // 2 - 1:
                            pool_x.release()
                            pool_y.release()
            for it in pending:
                emit_ctx(*it)
            emit_out_tail()

    nc.compile()
    return nc


def _get_nc():
    global _CACHED_NC
    if _CACHED_NC is None:
        _CACHED_NC = _build_nc()
    return _CACHED_NC


def _prep_inputs(states, key_states, masks, attention_bias, Wq, Wk, Wv, Wout,
                 bias_embs, bias_scalar):
    bf = ml_dtypes.bfloat16
    states = np.asarray(states, dtype=np.float32)
    key_states = np.asarray(key_states, dtype=np.float32)
    masks = np.asarray(masks, dtype=np.float32)
    ab = np.asarray(attention_bias)
    Wq2 = np.asarray(Wq, dtype=np.float32).reshape(D, HA)
    Wk3 = np.asarray(Wk, dtype=np.float32)
    Wv2 = np.asarray(Wv, dtype=np.float32).reshape(D, HA)
    Wout2 = np.asarray(Wout, dtype=np.float32).reshape(HA, D)
    bias_embs = np.asarray(bias_embs, dtype=np.float32)
    bias_scalar = np.asarray(bias_scalar, dtype=np.float32)

    bvals = (bias_embs[ab[:, 0]] @ bias_scalar)[:, 0]          # [E]
    # weight tile layouts: [m, D, 128] column blocks
    wqt_b = np.ascontiguousarray(
        Wq2.reshape(D, KD, P).transpose(1, 0, 2)).astype(bf)
    wkt_b = np.ascontiguousarray(
        Wk3.reshape(D, HA).reshape(D, KD, P).transpose(1, 0, 2)).astype(bf)
    wv_b = np.ascontiguousarray(Wv2).astype(bf)
    wks_b = np.ascontiguousarray(Wk3.sum(axis=2)).astype(bf)   # [D, H]
    wot_b = np.ascontiguousarray(
        Wout2.reshape(HA, KD, P).transpose(1, 0, 2)).astype(bf)

    in_maps = []
    for b in range(B):
        bs = np.zeros((T, T), dtype=np.float32)
        sel = ab[:, 1] == b
        bs[ab[sel, 2], ab[sel, 3]] = bvals[sel]                # last write wins
        in_maps.append({
            "xT": np.ascontiguousarray(states[b].T).astype(bf),
            "yT": np.ascontiguousarray(key_states[b].T).astype(bf),
            "wqt": wqt_b, "wkt": wkt_b, "wv": wv_b, "wks": wks_b,
            "wot": wot_b,
            "bsm": np.ascontiguousarray(bs.T).astype(bf),
            "mneg": np.ascontiguousarray(masks[b].T * MASK_NEG).astype(bf),
        })
    return in_maps


def kernel(**inputs) -> np.ndarray:
    nc = _get_nc()
    in_maps = _prep_inputs(**inputs)
    res = run_bass_kernel_spmd(nc, in_maps, core_ids=list(range(8)))
    out = np.empty((B, T, D), dtype=np.float32)
    for b in range(B):
        out[b] = res.results[b]["out"].T
    return out


# revision 24
# speedup vs baseline: 1.2732x; 1.0052x over previous
"""Sparse-attention layer on 8 TRN2 NeuronCores (data-parallel over batch).

Reference computation (per batch b):
    q = states @ Wq; k = key @ Wk; v = key @ Wv            [T, H, A]
    alpha[h,q,k] = q.k + bs[q,k]*ksum[k,h]                 (bs = sparse edge bias scatter)
    alpha = alpha/8 - mask*BIG; P = softmax_k(alpha)
    out = (P @ v) @ Wout                                   [T, D]

Device strategy (one batch per core, no collectives):
  - scores are computed TRANSPOSED, S^T[k,q], so the bias term bs[q,k]*ksum[k,h]
    becomes a per-partition scalar multiply -> one fused DVE scalar_tensor_tensor
    (bias apply + PSUM evacuation + bf16 cast in a single pass).
  - exp without max-subtraction (scores are O(20); fp32 exp range is ample);
    mask enters as an additive -30000 before the exp.
  - context matmul carries a fused ones-column producing softmax denominators;
    per-iteration ctx bursts (no DVE deps) keep the PE clock gate warm.
  - projections are streamed just-in-time inside the attention loop so the
    DVE (the critical engine) starts within ~15us of kernel start.
  - output projection for the first query half overlaps the second half's
    attention; host transposes the [D,T] result back.
"""

import sys

sys.path.insert(0, "/opt/trn_rl_repo")

import ml_dtypes
import numpy as np

import concourse.bass as bass
import concourse.tile as tile
from concourse import bacc, mybir
from concourse.bass_utils import run_bass_kernel_spmd

BF16 = mybir.dt.bfloat16
F32 = mybir.dt.float32
MULT = mybir.AluOpType.mult
ADD = mybir.AluOpType.add
EXP = mybir.ActivationFunctionType.Exp

B, T, D, H, A = 8, 1024, 1024, 16, 64
HA = H * A
P = 128
KD = D // P      # contraction tiles over D
KT = T // P      # tiles over key tokens
NQ = 2           # query-token 512-chunks
NC_ = 512
MASK_NEG = -30000.0

_CACHED_NC = None


def _build_nc():
    nc = bacc.Bacc("TRN2", target_bir_lowering=False, debug=False, num_devices=8)

    xT = nc.dram_tensor("xT", [D, T], BF16, kind="ExternalInput")
    yT = nc.dram_tensor("yT", [D, T], BF16, kind="ExternalInput")
    wqt = nc.dram_tensor("wqt", [KD, D, P], BF16, kind="ExternalInput")
    wkt_d = nc.dram_tensor("wkt", [KD, D, P], BF16, kind="ExternalInput")
    wv = nc.dram_tensor("wv", [D, HA], BF16, kind="ExternalInput")
    wks = nc.dram_tensor("wks", [D, H], BF16, kind="ExternalInput")
    wot = nc.dram_tensor("wot", [KD, HA, P], BF16, kind="ExternalInput")
    bsm = nc.dram_tensor("bsm", [T, T], BF16, kind="ExternalInput")
    mneg = nc.dram_tensor("mneg", [T, T], BF16, kind="ExternalInput")
    out = nc.dram_tensor("out", [D, T], F32, kind="ExternalOutput")

    with tile.TileContext(nc) as tc:
        with tc.tile_pool(name="persist", bufs=1) as pp, \
             tc.tile_pool(name="dscr", bufs=1, space="DRAM") as dpool, \
             tc.tile_pool(name="pb", bufs=1) as pb, \
             tc.tile_pool(name="ptmp", bufs=6) as ptmp, \
             tc.tile_pool(name="prst", bufs=2) as prst, \
             tc.tile_pool(name="pblk", bufs=4) as pblk, \
             tc.tile_pool(name="pwqk", bufs=2) as pwqk, \
             tc.tile_pool(name="pqk", bufs=3) as pqk, \
             tc.tile_pool(name="rbp", bufs=4) as rbp, \
             tc.tile_pool(name="po", bufs=2) as po, \
             tc.tile_pool(name="pwom", bufs=2) as pwom, \
             tc.tile_pool(name="sps", bufs=4, space="PSUM") as spsum, \
             tc.tile_pool(name="cps", bufs=2, space="PSUM") as cpsum, \
             tc.tile_pool(name="aps", bufs=2, space="PSUM") as apsum:
            v_sb = [pp.tile([P, H, A + 1], BF16, tag=f"v{i}", name=f"v{i}")
                    for i in range(KT)]
            ksum = pp.tile([P, KT * H], F32, tag="ksum", name="ksum")
            ctxT = [pp.tile([P, T], BF16, tag=f"ctx{i}", name=f"ctx{i}")
                    for i in range(KT)]
            rs = pp.tile([4 * H, NC_], F32, tag="rs", name="rs")  # row n*32+h
            rsr = pp.tile([4 * H, NC_], F32, tag="rsr", name="rsr")
            scr = dpool.tile([4 * H, NC_], F32, name="scr")

            # ---- loads for the prologue (key side first) ----
            pool_y = tc.alloc_tile_pool(name="py", bufs=1)
            yTs = [pool_y.tile([P, T], BF16, tag=f"yTs{i}", name=f"yTs{i}")
                   for i in range(KD)]
            wkss = pool_y.tile([P, KD * H], BF16, tag="wkss", name="wkss")
            for i in range(KD):
                sl = slice(i * P, (i + 1) * P)
                nc.sync.dma_start(yTs[i][:], yT.ap()[sl, :])
                nc.sync.dma_start(wkss[:, i * H:(i + 1) * H], wks.ap()[sl, :])
            pool_x = tc.alloc_tile_pool(name="px", bufs=1)
            xTs = [pool_x.tile([P, T], BF16, tag=f"xTs{i}", name=f"xTs{i}")
                   for i in range(KD)]
            for i in range(KD):
                nc.sync.dma_start(xTs[i][:], xT.ap()[i * P:(i + 1) * P, :])
            pool_v = tc.alloc_tile_pool(name="pv", bufs=1)

            # ---- ksum (needed by every score tile) ----
            for m in range(KT):
                msl = slice(m * P, (m + 1) * P)
                ps = apsum.tile([P, NC_], F32, tag="aps", name="aps")
                for kd in range(KD):
                    nc.tensor.matmul(ps[:, 0:H], yTs[kd][:, msl],
                                     wkss[:, kd * H:(kd + 1) * H],
                                     start=(kd == 0), stop=(kd == KD - 1))
                nc.vector.tensor_copy(ksum[:, m * H:(m + 1) * H], ps[:, 0:H])

            def emit_kT(m):
                kTr = pqk.tile([P, T], BF16, tag="kTr", name="kTr")
                wkm = pwqk.tile([P, KD, P], BF16, tag="wkm", name="wkm")
                for kd in range(KD):
                    nc.sync.dma_start(wkm[:, kd, :],
                                      wkt_d.ap()[m, kd * P:(kd + 1) * P, :])
                for n in range(NQ):
                    nsl = slice(n * NC_, (n + 1) * NC_)
                    ps = apsum.tile([P, NC_], F32, tag="aps", name="aps")
                    for kd in range(KD):
                        nc.tensor.matmul(ps[:], wkm[:, kd, :],
                                         yTs[kd][:, nsl],
                                         start=(kd == 0), stop=(kd == KD - 1))
                    nc.scalar.copy(kTr[:, nsl], ps[:])
                return kTr

            def emit_qT(m):
                qTr = pqk.tile([P, T], BF16, tag="qTr", name="qTr")
                wqm = pwqk.tile([P, KD, P], BF16, tag="wqm", name="wqm")
                for kd in range(KD):
                    nc.sync.dma_start(wqm[:, kd, :],
                                      wqt.ap()[m, kd * P:(kd + 1) * P, :])
                for n in range(NQ):
                    nsl = slice(n * NC_, (n + 1) * NC_)
                    ps = apsum.tile([P, NC_], F32, tag="aps", name="aps")
                    for kd in range(KD):
                        nc.tensor.matmul(ps[:], wqm[:, kd, :],
                                         xTs[kd][:, nsl],
                                         start=(kd == 0), stop=(kd == KD - 1))
                    nc.scalar.copy(qTr[:, nsl], ps[:])
                return qTr

            def emit_v(m):
                msl = slice(m * P, (m + 1) * P)
                nc.gpsimd.memset(v_sb[m][:, :, A:A + 1], 1.0)
                for n in range(NQ):
                    nsl = slice(n * NC_, (n + 1) * NC_)
                    ps = apsum.tile([P, NC_], F32, tag="aps", name="aps")
                    for kd in range(KD):
                        nc.tensor.matmul(ps[:], yTs[kd][:, msl],
                                         wvt[kd][:, nsl],
                                         start=(kd == 0), stop=(kd == KD - 1))
                    nc.scalar.copy(
                        v_sb[m][:, n * (H // 2):(n + 1) * (H // 2), 0:A],
                        ps[:].rearrange("p (h a) -> p h a", a=A))

            def emit_scores(hp, n, kTr, qTr):
                nsl = slice(n * NC_, (n + 1) * NC_)
                pblks = [pblk.tile([P, KT, NC_], BF16, tag="Pblk", name="Pblk")
                         for _ in range(2)]
                for kt in range(KT):
                    for hi in range(2):
                        h = 2 * hp + hi
                        roff = hi * A
                        sps = spsum.tile([P, NC_], F32, tag="sps", name="sps")
                        nc.tensor.matmul(
                            sps[:], kTr[roff:roff + A, kt * P:(kt + 1) * P],
                            qTr[roff:roff + A, nsl], start=True, stop=True)
                        s1 = ptmp.tile([P, NC_], BF16, tag="s1", name="s1")
                        nc.vector.scalar_tensor_tensor(
                            s1[:], bsm_sb[kt][:, nsl],
                            ksum[:, kt * H + h:kt * H + h + 1],
                            sps[:], op0=MULT, op1=ADD)
                        nc.vector.tensor_tensor(
                            pblks[hi][:, kt, :], s1[:],
                            mneg_n[n][:, kt, :], op=ADD)
                for hi in range(2):
                    nc.scalar.activation(pblks[hi][:], pblks[hi][:], EXP,
                                         scale=0.125)
                return pblks

            def emit_ctx(hp, n, pblks):
                nsl = slice(n * NC_, (n + 1) * NC_)
                for hi in range(2):
                    h = 2 * hp + hi
                    roff = hi * A
                    cps = cpsum.tile([A + 1, NC_], F32, tag="cps", name="cps")
                    for kt in range(KT):
                        nc.tensor.matmul(
                            cps[:], v_sb[kt][:, h, :], pblks[hi][:, kt, :],
                            start=(kt == 0), stop=(kt == KT - 1))
                    r = n * 2 * H + h
                    rstage = prst.tile([1, NC_], F32, tag="rstage",
                                       name="rstage")
                    nc.scalar.copy(rstage[:], cps[A:A + 1, :])
                    nc.sync.dma_start(rs[r:r + 1, :], rstage[:])
                    nc.scalar.copy(ctxT[hp][roff:roff + A, nsl], cps[0:A, :])

            def emit_out_tail():
                for n in range(NQ):
                    rsl = slice(n * 2 * H, n * 2 * H + H)
                    nc.vector.reciprocal(rsr[rsl, :], rs[rsl, :])
                    nc.sync.dma_start(scr[rsl, :], rsr[rsl, :])
                # normalize all chunks; rb cast to bf16 so the multiply is 2x
                for n in range(NQ):
                    nsl = slice(n * NC_, (n + 1) * NC_)
                    for hp in range(H // 2):
                        r0 = n * 2 * H + 2 * hp
                        r1 = n * 2 * H + 2 * hp + 1
                        rb = rbp.tile([P, NC_], BF16, tag="rb", name="rb")
                        src0 = bass.AP(scr[:].tensor, scr[:].offset + r0 * NC_,
                                       [[0, A], [1, NC_]])
                        src1 = bass.AP(scr[:].tensor, scr[:].offset + r1 * NC_,
                                       [[0, A], [1, NC_]])
                        nc.gpsimd.dma_start(rb[0:A, :], src0)
                        nc.gpsimd.dma_start(rb[A:P, :], src1)
                        nc.vector.tensor_tensor(ctxT[hp][:, nsl],
                                                ctxT[hp][:, nsl], rb[:],
                                                op=MULT)
                for m in range(KD):
                    msl = slice(m * P, (m + 1) * P)
                    wom = pwom.tile([P, KD, P], BF16, tag="wom", name="wom")
                    for kd in range(KD):
                        nc.sync.dma_start(
                            wom[:, kd, :],
                            wot.ap()[m, kd * P:(kd + 1) * P, :])
                    for n in range(NQ):
                        nsl = slice(n * NC_, (n + 1) * NC_)
                        ps = apsum.tile([P, NC_], F32, tag="aps", name="aps")
                        for kt in range(KT):
                            nc.tensor.matmul(ps[:], wom[:, kt, :],
                                             ctxT[kt][:, nsl],
                                             start=(kt == 0),
                                             stop=(kt == KT - 1))
                        osb = po.tile([P, NC_], F32, tag="osb", name="osb")
                        nc.scalar.copy(osb[:], ps[:])
                        nc.sync.dma_start(out.ap()[msl, nsl], osb[:])

            cur_k = emit_kT(0)
            cur_q = emit_qT(0)

            # bulk loads: needed from the first stt (bsm), first adds (mneg),
            # and the v projections emitted during hp=0/1
            bsm_sb = [pb.tile([P, T], BF16, tag=f"bsm{i}", name=f"bsm{i}")
                      for i in range(KT)]
            mneg_n = [pb.tile([P, KT, NC_], BF16, tag=f"mnegn{n}",
                              name=f"mnegn{n}") for n in range(NQ)]
            for i in range(KT):
                sl = slice(i * P, (i + 1) * P)
                nc.sync.dma_start(bsm_sb[i][:], bsm.ap()[sl, :])
                for n in range(NQ):
                    nc.sync.dma_start(mneg_n[n][:, i, :],
                                      mneg.ap()[sl, n * NC_:(n + 1) * NC_])
            wvt = [pool_v.tile([P, HA], BF16, tag=f"wvt{i}", name=f"wvt{i}")
                   for i in range(KD)]
            for i in range(KD):
                nc.sync.dma_start(wvt[i][:], wv.ap()[i * P:(i + 1) * P, :])

            pending = []
            for hp in range(H // 2):
                for n in range(NQ):
                    if hp == 1 and n == 0:
                        emit_v(6)
                        emit_v(7)
                        pool_v.release()
                    if len(pending) >= 2:
                        emit_ctx(*pending.pop(0))
                    pblks = emit_scores(hp, n, cur_k, cur_q)
                    pending.append((hp, n, pblks))
                    if n == 0:
                        if hp == 0:
                            for m in range(3):
                                emit_v(m)
                    else:
                        if hp == 0:
                            for m in range(3, 6):
                                emit_v(m)
                        if hp < H // 2 - 1:
                            cur_k = emit_kT(hp + 1)
                            cur_q = emit_qT(hp + 1)
                        if hp == H // 2 - 1:
                            pool_x.release()
                            pool_y.release()
            for it in pending:
                emit_ctx(*it)
            emit_out_tail()

    nc.compile()
    return nc


def _get_nc():
    global _CACHED_NC
    if _CACHED_NC is None:
        _CACHED_NC = _build_nc()
    return _CACHED_NC


def _prep_inputs(states, key_states, masks, attention_bias, Wq, Wk, Wv, Wout,
                 bias_embs, bias_scalar):
    bf = ml_dtypes.bfloat16
    states = np.asarray(states, dtype=np.float32)
    key_states = np.asarray(key_states, dtype=np.float32)
    masks = np.asarray(masks, dtype=np.float32)
    ab = np.asarray(attention_bias)
    Wq2 = np.asarray(Wq, dtype=np.float32).reshape(D, HA)
    Wk3 = np.asarray(Wk, dtype=np.float32)
    Wv2 = np.asarray(Wv, dtype=np.float32).reshape(D, HA)
    Wout2 = np.asarray(Wout, dtype=np.float32).reshape(HA, D)
    bias_embs = np.asarray(bias_embs, dtype=np.float32)
    bias_scalar = np.asarray(bias_scalar, dtype=np.float32)

    bvals = (bias_embs[ab[:, 0]] @ bias_scalar)[:, 0]          # [E]
    # weight tile layouts: [m, D, 128] column blocks
    wqt_b = np.ascontiguousarray(
        Wq2.reshape(D, KD, P).transpose(1, 0, 2)).astype(bf)
    wkt_b = np.ascontiguousarray(
        Wk3.reshape(D, HA).reshape(D, KD, P).transpose(1, 0, 2)).astype(bf)
    wv_b = np.ascontiguousarray(Wv2).astype(bf)
    wks_b = np.ascontiguousarray(Wk3.sum(axis=2)).astype(bf)   # [D, H]
    wot_b = np.ascontiguousarray(
        Wout2.reshape(HA, KD, P).transpose(1, 0, 2)).astype(bf)

    in_maps = []
    for b in range(B):
        bs = np.zeros((T, T), dtype=np.float32)
        sel = ab[:, 1] == b
        bs[ab[sel, 2], ab[sel, 3]] = bvals[sel]                # last write wins
        in_maps.append({
            "xT": np.ascontiguousarray(states[b].T).astype(bf),
            "yT": np.ascontiguousarray(key_states[b].T).astype(bf),
            "wqt": wqt_b, "wkt": wkt_b, "wv": wv_b, "wks": wks_b,
            "wot": wot_b,
            "bsm": np.ascontiguousarray(bs.T).astype(bf),
            "mneg": np.ascontiguousarray(masks[b].T * MASK_NEG).astype(bf),
        })
    return in_maps


def kernel(**inputs) -> np.ndarray:
    nc = _get_nc()
    in_maps = _prep_inputs(**inputs)
    res = run_bass_kernel_spmd(nc, in_maps, core_ids=list(range(8)))
    out = np.empty((B, T, D), dtype=np.float32)
    for b in range(B):
        out[b] = res.results[b]["out"].T
    return out


# revision 25
# speedup vs baseline: 1.2865x; 1.0104x over previous
"""Sparse-attention layer on 8 TRN2 NeuronCores (data-parallel over batch).

Reference computation (per batch b):
    q = states @ Wq; k = key @ Wk; v = key @ Wv            [T, H, A]
    alpha[h,q,k] = q.k + bs[q,k]*ksum[k,h]                 (bs = sparse edge bias scatter)
    alpha = alpha/8 - mask*BIG; P = softmax_k(alpha)
    out = (P @ v) @ Wout                                   [T, D]

Device strategy (one batch per core, no collectives):
  - scores are computed TRANSPOSED, S^T[k,q], so the bias term bs[q,k]*ksum[k,h]
    becomes a per-partition scalar multiply -> one fused DVE scalar_tensor_tensor
    (bias apply + PSUM evacuation + bf16 cast in a single pass).
  - exp without max-subtraction (scores are O(20); fp32 exp range is ample);
    mask enters as an additive -30000 before the exp.
  - context matmul carries a fused ones-column producing softmax denominators;
    per-iteration ctx bursts (no DVE deps) keep the PE clock gate warm.
  - projections are streamed just-in-time inside the attention loop so the
    DVE (the critical engine) starts within ~15us of kernel start.
  - output projection for the first query half overlaps the second half's
    attention; host transposes the [D,T] result back.
"""

import sys

sys.path.insert(0, "/opt/trn_rl_repo")

import ml_dtypes
import numpy as np

import concourse.bass as bass
import concourse.tile as tile
from concourse import bacc, mybir
from concourse.bass_utils import run_bass_kernel_spmd

BF16 = mybir.dt.bfloat16
F32 = mybir.dt.float32
MULT = mybir.AluOpType.mult
ADD = mybir.AluOpType.add
EXP = mybir.ActivationFunctionType.Exp

B, T, D, H, A = 8, 1024, 1024, 16, 64
HA = H * A
P = 128
KD = D // P      # contraction tiles over D
KT = T // P      # tiles over key tokens
NQ = 2           # query-token 512-chunks
NC_ = 512
MASK_NEG = -30000.0

_CACHED_NC = None


def _build_nc():
    nc = bacc.Bacc("TRN2", target_bir_lowering=False, debug=False, num_devices=8)

    xT = nc.dram_tensor("xT", [D, T], BF16, kind="ExternalInput")
    yT = nc.dram_tensor("yT", [D, T], BF16, kind="ExternalInput")
    wqt = nc.dram_tensor("wqt", [KD, D, P], BF16, kind="ExternalInput")
    wkt_d = nc.dram_tensor("wkt", [KD, D, P], BF16, kind="ExternalInput")
    wv = nc.dram_tensor("wv", [D, HA], BF16, kind="ExternalInput")
    wks = nc.dram_tensor("wks", [D, H], BF16, kind="ExternalInput")
    wot = nc.dram_tensor("wot", [KD, HA, P], BF16, kind="ExternalInput")
    bsm = nc.dram_tensor("bsm", [T, T], BF16, kind="ExternalInput")
    mneg = nc.dram_tensor("mneg", [T, T], BF16, kind="ExternalInput")
    out = nc.dram_tensor("out", [D, T], F32, kind="ExternalOutput")

    with tile.TileContext(nc) as tc:
        with tc.tile_pool(name="persist", bufs=1) as pp, \
             tc.tile_pool(name="dscr", bufs=1, space="DRAM") as dpool, \
             tc.tile_pool(name="pb", bufs=1) as pb, \
             tc.tile_pool(name="ptmp", bufs=6) as ptmp, \
             tc.tile_pool(name="prst", bufs=2) as prst, \
             tc.tile_pool(name="pblk", bufs=4) as pblk, \
             tc.tile_pool(name="pwqk", bufs=2) as pwqk, \
             tc.tile_pool(name="pqk", bufs=3) as pqk, \
             tc.tile_pool(name="rbp", bufs=4) as rbp, \
             tc.tile_pool(name="po", bufs=2) as po, \
             tc.tile_pool(name="pwom", bufs=2) as pwom, \
             tc.tile_pool(name="sps", bufs=4, space="PSUM") as spsum, \
             tc.tile_pool(name="cps", bufs=2, space="PSUM") as cpsum, \
             tc.tile_pool(name="aps", bufs=2, space="PSUM") as apsum:
            v_sb = [pp.tile([P, H, A + 1], BF16, tag=f"v{i}", name=f"v{i}")
                    for i in range(KT)]
            ksum = pp.tile([P, KT * H], F32, tag="ksum", name="ksum")
            ctxT = [pp.tile([P, T], BF16, tag=f"ctx{i}", name=f"ctx{i}")
                    for i in range(KT)]
            rs = pp.tile([4 * H, NC_], F32, tag="rs", name="rs")  # row n*32+h
            rsr = pp.tile([4 * H, NC_], F32, tag="rsr", name="rsr")
            scr = dpool.tile([4 * H, NC_], F32, name="scr")

            # ---- loads for the prologue (key side first) ----
            pool_y = tc.alloc_tile_pool(name="py", bufs=1)
            yTs = [pool_y.tile([P, T], BF16, tag=f"yTs{i}", name=f"yTs{i}")
                   for i in range(KD)]
            wkss = pool_y.tile([P, KD * H], BF16, tag="wkss", name="wkss")
            for i in range(KD):
                sl = slice(i * P, (i + 1) * P)
                nc.sync.dma_start(yTs[i][:], yT.ap()[sl, :])
                nc.sync.dma_start(wkss[:, i * H:(i + 1) * H], wks.ap()[sl, :])
            pool_x = tc.alloc_tile_pool(name="px", bufs=1)
            xTs = [pool_x.tile([P, T], BF16, tag=f"xTs{i}", name=f"xTs{i}")
                   for i in range(KD)]
            for i in range(KD):
                nc.sync.dma_start(xTs[i][:], xT.ap()[i * P:(i + 1) * P, :])
            pool_v = tc.alloc_tile_pool(name="pv", bufs=1)

            # ---- ksum (needed by every score tile) ----
            for m in range(KT):
                msl = slice(m * P, (m + 1) * P)
                ps = apsum.tile([P, NC_], F32, tag="aps", name="aps")
                for kd in range(KD):
                    nc.tensor.matmul(ps[:, 0:H], yTs[kd][:, msl],
                                     wkss[:, kd * H:(kd + 1) * H],
                                     start=(kd == 0), stop=(kd == KD - 1))
                nc.vector.tensor_copy(ksum[:, m * H:(m + 1) * H], ps[:, 0:H])

            def emit_kT(m):
                kTr = pqk.tile([P, T], BF16, tag="kTr", name="kTr")
                wkm = pwqk.tile([P, KD, P], BF16, tag="wkm", name="wkm")
                for kd in range(KD):
                    nc.sync.dma_start(wkm[:, kd, :],
                                      wkt_d.ap()[m, kd * P:(kd + 1) * P, :])
                for n in range(NQ):
                    nsl = slice(n * NC_, (n + 1) * NC_)
                    ps = apsum.tile([P, NC_], F32, tag="aps", name="aps")
                    for kd in range(KD):
                        nc.tensor.matmul(ps[:], wkm[:, kd, :],
                                         yTs[kd][:, nsl],
                                         start=(kd == 0), stop=(kd == KD - 1))
                    nc.scalar.copy(kTr[:, nsl], ps[:])
                return kTr

            def emit_qT(m):
                qTr = pqk.tile([P, T], BF16, tag="qTr", name="qTr")
                wqm = pwqk.tile([P, KD, P], BF16, tag="wqm", name="wqm")
                for kd in range(KD):
                    nc.sync.dma_start(wqm[:, kd, :],
                                      wqt.ap()[m, kd * P:(kd + 1) * P, :])
                for n in range(NQ):
                    nsl = slice(n * NC_, (n + 1) * NC_)
                    ps = apsum.tile([P, NC_], F32, tag="aps", name="aps")
                    for kd in range(KD):
                        nc.tensor.matmul(ps[:], wqm[:, kd, :],
                                         xTs[kd][:, nsl],
                                         start=(kd == 0), stop=(kd == KD - 1))
                    nc.scalar.copy(qTr[:, nsl], ps[:])
                return qTr

            def emit_v(m):
                msl = slice(m * P, (m + 1) * P)
                nc.gpsimd.memset(v_sb[m][:, :, A:A + 1], 1.0)
                for n in range(NQ):
                    nsl = slice(n * NC_, (n + 1) * NC_)
                    ps = apsum.tile([P, NC_], F32, tag="aps", name="aps")
                    for kd in range(KD):
                        nc.tensor.matmul(ps[:], yTs[kd][:, msl],
                                         wvt[kd][:, nsl],
                                         start=(kd == 0), stop=(kd == KD - 1))
                    nc.scalar.copy(
                        v_sb[m][:, n * (H // 2):(n + 1) * (H // 2), 0:A],
                        ps[:].rearrange("p (h a) -> p h a", a=A))

            def emit_scores(hp, n, kTr, qTr):
                nsl = slice(n * NC_, (n + 1) * NC_)
                pblks = [pblk.tile([P, KT, NC_], BF16, tag="Pblk", name="Pblk")
                         for _ in range(2)]
                for kt in range(KT):
                    for hi in range(2):
                        h = 2 * hp + hi
                        roff = hi * A
                        sps = spsum.tile([P, NC_], F32, tag="sps", name="sps")
                        nc.tensor.matmul(
                            sps[:], kTr[roff:roff + A, kt * P:(kt + 1) * P],
                            qTr[roff:roff + A, nsl], start=True, stop=True)
                        s1 = ptmp.tile([P, NC_], BF16, tag="s1", name="s1")
                        nc.vector.scalar_tensor_tensor(
                            s1[:], bsm_sb[kt][:, nsl],
                            ksum[:, kt * H + h:kt * H + h + 1],
                            sps[:], op0=MULT, op1=ADD)
                        nc.vector.tensor_tensor(
                            pblks[hi][:, kt, :], s1[:],
                            mneg_n[n][:, kt, :], op=ADD)
                for hi in range(2):
                    nc.scalar.activation(pblks[hi][:], pblks[hi][:], EXP,
                                         scale=0.125)
                return pblks

            def emit_ctx(hp, n, pblks):
                nsl = slice(n * NC_, (n + 1) * NC_)
                for hi in range(2):
                    h = 2 * hp + hi
                    roff = hi * A
                    cps = cpsum.tile([A + 1, NC_], F32, tag="cps", name="cps")
                    for kt in range(KT):
                        nc.tensor.matmul(
                            cps[:], v_sb[kt][:, h, :], pblks[hi][:, kt, :],
                            start=(kt == 0), stop=(kt == KT - 1))
                    r = n * 2 * H + h
                    rstage = prst.tile([1, NC_], F32, tag="rstage",
                                       name="rstage")
                    nc.scalar.copy(rstage[:], cps[A:A + 1, :])
                    nc.sync.dma_start(rs[r:r + 1, :], rstage[:])
                    nc.scalar.copy(ctxT[hp][roff:roff + A, nsl], cps[0:A, :])

            def emit_out_tail():
                for n in range(NQ):
                    rsl = slice(n * 2 * H, n * 2 * H + H)
                    nc.vector.reciprocal(rsr[rsl, :], rs[rsl, :])
                    nc.sync.dma_start(scr[rsl, :], rsr[rsl, :])
                # normalize all chunks; rb cast to bf16 so the multiply is 2x
                for n in range(NQ):
                    nsl = slice(n * NC_, (n + 1) * NC_)
                    for hp in range(H // 2):
                        r0 = n * 2 * H + 2 * hp
                        r1 = n * 2 * H + 2 * hp + 1
                        rb = rbp.tile([P, NC_], F32, tag="rb", name="rb")
                        src0 = bass.AP(scr[:].tensor, scr[:].offset + r0 * NC_,
                                       [[0, A], [1, NC_]])
                        src1 = bass.AP(scr[:].tensor, scr[:].offset + r1 * NC_,
                                       [[0, A], [1, NC_]])
                        nc.sync.dma_start(rb[0:A, :], src0)
                        nc.sync.dma_start(rb[A:P, :], src1)
                        nc.vector.tensor_tensor(ctxT[hp][:, nsl],
                                                ctxT[hp][:, nsl], rb[:],
                                                op=MULT)
                for m in range(KD):
                    msl = slice(m * P, (m + 1) * P)
                    wom = pwom.tile([P, KD, P], BF16, tag="wom", name="wom")
                    for kd in range(KD):
                        nc.sync.dma_start(
                            wom[:, kd, :],
                            wot.ap()[m, kd * P:(kd + 1) * P, :])
                    for n in range(NQ):
                        nsl = slice(n * NC_, (n + 1) * NC_)
                        ps = apsum.tile([P, NC_], F32, tag="aps", name="aps")
                        for kt in range(KT):
                            nc.tensor.matmul(ps[:], wom[:, kt, :],
                                             ctxT[kt][:, nsl],
                                             start=(kt == 0),
                                             stop=(kt == KT - 1))
                        osb = po.tile([P, NC_], F32, tag="osb", name="osb")
                        nc.scalar.copy(osb[:], ps[:])
                        nc.sync.dma_start(out.ap()[msl, nsl], osb[:])

            cur_k = emit_kT(0)
            cur_q = emit_qT(0)

            # bulk loads: needed from the first stt (bsm), first adds (mneg),
            # and the v projections emitted during hp=0/1
            bsm_sb = [pb.tile([P, T], BF16, tag=f"bsm{i}", name=f"bsm{i}")
                      for i in range(KT)]
            mneg_n = [pb.tile([P, KT, NC_], BF16, tag=f"mnegn{n}",
                              name=f"mnegn{n}") for n in range(NQ)]
            for i in range(KT):
                sl = slice(i * P, (i + 1) * P)
                nc.sync.dma_start(bsm_sb[i][:], bsm.ap()[sl, :])
                for n in range(NQ):
                    nc.sync.dma_start(mneg_n[n][:, i, :],
                                      mneg.ap()[sl, n * NC_:(n + 1) * NC_])
            wvt = [pool_v.tile([P, HA], BF16, tag=f"wvt{i}", name=f"wvt{i}")
                   for i in range(KD)]
            for i in range(KD):
                nc.sync.dma_start(wvt[i][:], wv.ap()[i * P:(i + 1) * P, :])

            pending = []
            for hp in range(H // 2):
                for n in range(NQ):
                    if hp == 1 and n == 0:
                        emit_v(6)
                        emit_v(7)
                        pool_v.release()
                    if len(pending) >= 2:
                        emit_ctx(*pending.pop(0))
                    pblks = emit_scores(hp, n, cur_k, cur_q)
                    pending.append((hp, n, pblks))
                    if n == 0:
                        if hp == 0:
                            for m in range(3):
                                emit_v(m)
                    else:
                        if hp == 0:
                            for m in range(3, 6):
                                emit_v(m)
                        if hp < H // 2 - 1:
                            cur_k = emit_kT(hp + 1)
                            cur_q = emit_qT(hp + 1)
                        if hp == H // 2 - 1:
                            pool_x.release()
                            pool_y.release()
            for it in pending:
                emit_ctx(*it)
            emit_out_tail()

    nc.compile()
    return nc


def _get_nc():
    global _CACHED_NC
    if _CACHED_NC is None:
        _CACHED_NC = _build_nc()
    return _CACHED_NC


def _prep_inputs(states, key_states, masks, attention_bias, Wq, Wk, Wv, Wout,
                 bias_embs, bias_scalar):
    bf = ml_dtypes.bfloat16
    states = np.asarray(states, dtype=np.float32)
    key_states = np.asarray(key_states, dtype=np.float32)
    masks = np.asarray(masks, dtype=np.float32)
    ab = np.asarray(attention_bias)
    Wq2 = np.asarray(Wq, dtype=np.float32).reshape(D, HA)
    Wk3 = np.asarray(Wk, dtype=np.float32)
    Wv2 = np.asarray(Wv, dtype=np.float32).reshape(D, HA)
    Wout2 = np.asarray(Wout, dtype=np.float32).reshape(HA, D)
    bias_embs = np.asarray(bias_embs, dtype=np.float32)
    bias_scalar = np.asarray(bias_scalar, dtype=np.float32)

    bvals = (bias_embs[ab[:, 0]] @ bias_scalar)[:, 0]          # [E]
    # weight tile layouts: [m, D, 128] column blocks
    wqt_b = np.ascontiguousarray(
        Wq2.reshape(D, KD, P).transpose(1, 0, 2)).astype(bf)
    wkt_b = np.ascontiguousarray(
        Wk3.reshape(D, HA).reshape(D, KD, P).transpose(1, 0, 2)).astype(bf)
    wv_b = np.ascontiguousarray(Wv2).astype(bf)
    wks_b = np.ascontiguousarray(Wk3.sum(axis=2)).astype(bf)   # [D, H]
    wot_b = np.ascontiguousarray(
        Wout2.reshape(HA, KD, P).transpose(1, 0, 2)).astype(bf)

    in_maps = []
    for b in range(B):
        bs = np.zeros((T, T), dtype=np.float32)
        sel = ab[:, 1] == b
        bs[ab[sel, 2], ab[sel, 3]] = bvals[sel]                # last write wins
        in_maps.append({
            "xT": np.ascontiguousarray(states[b].T).astype(bf),
            "yT": np.ascontiguousarray(key_states[b].T).astype(bf),
            "wqt": wqt_b, "wkt": wkt_b, "wv": wv_b, "wks": wks_b,
            "wot": wot_b,
            "bsm": np.ascontiguousarray(bs.T).astype(bf),
            "mneg": np.ascontiguousarray(masks[b].T * MASK_NEG).astype(bf),
        })
    return in_maps


def kernel(**inputs) -> np.ndarray:
    nc = _get_nc()
    in_maps = _prep_inputs(**inputs)
    res = run_bass_kernel_spmd(nc, in_maps, core_ids=list(range(8)))
    out = np.empty((B, T, D), dtype=np.float32)
    for b in range(B):
        out[b] = res.results[b]["out"].T
    return out
